# revision 1
# baseline (speedup 1.0000x reference)
"""NodeAttention GNN message passing kernel for 8 trn2 NeuronCores.

Problem (per batch element b, data-parallel over B=8 across 8 cores):
    s_nbr[j]  = features[j, :] @ w_att[:768]
    s_dep[i,j] = adj[i, j, :] @ w_att[768:832]
    mask[i,j] = any(adj[i,j,:] != 0)
    scores    = s_nbr[j] + s_dep[i,j] (+ s_asp[i], which cancels in softmax)
    w         = softmax_j(scores masked), zeroed off-mask
    agg       = w @ features
    out[i]    = (aspect[i] and any_j mask[i,j]) ? agg[i] : features[i]

Device pipeline per core (N=512 nodes, D=768, DEP=64):
  - stream adj in [128 i, 16 j * 64 k] tiles (f32r), j-chunk-major
  - PE transpose [128,128] blocks (2 j's x 64 k -> partitions) for all 4
    i-blocks -> PSUM [128, (dj,k)] x [4*128 i]
  - ACT copy PSUM->SBUF
  - scatter-matmul with a shifted two-column weight (w_dep | pair-select)
    accumulating s_dep^T[j', i] into one of 4 PSUM banks (j-blocks)
  - epilogue in the transposed domain: e^T = exp(s_dep^T + s_nbr[j]) masked
    by s_dep^T != 0; agg via matmul with e^T as stationary and
    [features | 1] as moving (ones column yields the softmax denominator);
    final per-row blend with features by the update mask.
"""
import sys

if "/opt/trn_rl_repo" not in sys.path:
    sys.path.insert(0, "/opt/trn_rl_repo")

import numpy as np
from contextlib import ExitStack

import concourse.bass as bass
from concourse import bacc
import concourse.mybir as mybir
import concourse.tile as tile
from concourse.bass_utils import run_bass_kernel_spmd

F32 = mybir.dt.float32
F32R = mybir.dt.float32r

N = 512     # nodes
D = 768     # feature dim
DEP = 64    # edge embedding dim
P = 128     # partitions
NB = N // P         # 4 node blocks
JC = 16             # j's per adj DMA tile
NJC = N // JC       # 32 j-chunks
JPAIRS = JC // 2    # 8 j-pairs per chunk

_CACHED = {}


def _build():
    nc = bacc.Bacc()
    adj = nc.dram_tensor("adj", [N, N * DEP], F32R, kind="ExternalInput")
    feat = nc.dram_tensor("feat", [N, D], F32, kind="ExternalInput")
    aspf = nc.dram_tensor("aspf", [N], F32, kind="ExternalInput")
    ident = nc.dram_tensor("ident", [P, P], F32R, kind="ExternalInput")
    wpad = nc.dram_tensor("wpad", [P, 126 + P], F32R, kind="ExternalInput")
    wnbr = nc.dram_tensor("wnbr", [D], F32, kind="ExternalInput")
    out = nc.dram_tensor("out", [N, D], F32, kind="ExternalOutput")

    with ExitStack() as ctx:
        tc = ctx.enter_context(tile.TileContext(nc))
        const = ctx.enter_context(tc.tile_pool(name="const", bufs=1))
        tpool = ctx.enter_context(tc.tile_pool(name="tpool", bufs=4))
        spool = ctx.enter_context(tc.tile_pool(name="spool", bufs=4))
        epool = ctx.enter_context(tc.tile_pool(name="epool", bufs=1))
        opool = ctx.enter_context(tc.tile_pool(name="opool", bufs=2))
        s_ps = ctx.enter_context(tc.tile_pool(name="s_ps", bufs=2, space="PSUM"))
        sd_ps = ctx.enter_context(tc.tile_pool(name="sd_ps", bufs=1, space="PSUM"))
        agg_ps = ctx.enter_context(tc.tile_pool(name="agg_ps", bufs=1, space="PSUM"))

        # ---- constants / small inputs ----
        ident_sb = const.tile([P, P], F32R)
        nc.sync.dma_start(ident_sb[:], ident[:, :])
        wpad_sb = const.tile([P, 126 + P], F32R)
        nc.sync.dma_start(wpad_sb[:], wpad[:, :])
        wnbr_sb = const.tile([P, D], F32)
        wnbr_ap = wnbr[:]
        nc.sync.dma_start(
            wnbr_sb[:],
            bass.AP(tensor=wnbr_ap.tensor, offset=wnbr_ap.offset,
                    ap=[[0, P]] + list(wnbr_ap.ap)),
        )

        # features with a ones column appended: [128, 769] per node block
        featp = []
        for b in range(NB):
            f = const.tile([P, D + 1], F32, tag=f"featp{b}", name=f"featp{b}")
            nc.sync.dma_start(f[:, 0:D], feat[b * P:(b + 1) * P, :])
            nc.vector.memset(f[:, D:D + 1], 1.0)
            featp.append(f)

        aspf_sb = const.tile([P, NB], F32)
        for b in range(NB):
            nc.sync.dma_start(
                aspf_sb[:, b:b + 1], aspf[b * P:(b + 1) * P],
            )

        # s_nbr[j] per node block: rowwise dot(features, w_nbr)
        snbr_sb = const.tile([P, NB], F32)
        for b in range(NB):
            fw = spool.tile([P, D], F32, tag="fw")
            nc.vector.tensor_mul(fw[:], featp[b][:, 0:D], wnbr_sb[:])
            nc.vector.tensor_reduce(
                snbr_sb[:, b:b + 1], fw[:],
                axis=mybir.AxisListType.X, op=mybir.AluOpType.add,
            )

        # ---- main stream: s_dep^T accumulation ----
        # SD[jb][j', i] = s_dep[i, jb*128 + j'] for all i
        sd = [sd_ps.tile([P, N], F32, tag=f"sd{jb}", name=f"sd{jb}") for jb in range(NB)]

        adj_v = adj.rearrange("(nb p) (jc f) -> nb p jc f", p=P, f=JC * DEP)

        for jc in range(NJC):
            jb = (jc * JC) // P
            tiles = []
            for b in range(NB):
                t = tpool.tile([P, JC * DEP], F32R, tag=f"t{b}", name=f"t{b}")
                nc.sync.dma_start(t[:], adj_v[b, :, jc, :])
                tiles.append(t)
            for tp in range(JPAIRS):
                m = jc * JPAIRS + tp          # global j-pair index 0..255
                mm = m % 64                   # pair index within j-block
                stage = s_ps.tile([P, N], F32R, tag="stage")
                for b in range(NB):
                    nc.tensor.transpose(
                        stage[:, b * P:(b + 1) * P],
                        tiles[b][:, 2 * tp * DEP:(2 * tp + 2) * DEP],
                        ident_sb[:],
                    )
                s_sb = spool.tile([P, N], F32R, tag="s_sb")
                nc.scalar.copy(s_sb[:], stage[:])
                nc.tensor.matmul(
                    sd[jb][:],
                    wpad_sb[:, 126 - 2 * mm:126 - 2 * mm + P],
                    s_sb[:],
                    start=(mm == 0),
                    stop=(mm == 63),
                )

        # ---- epilogue (transposed domain) ----
        # e_masked^T[j', i] = exp(sd + s_nbr[j]) * (sd != 0)
        em = []
        for jb in range(NB):
            e = epool.tile([P, N], F32, tag=f"e{jb}", name=f"e{jb}")
            nc.scalar.activation(
                e[:], sd[jb][:], mybir.ActivationFunctionType.Exp,
                bias=snbr_sb[:, jb:jb + 1], scale=1.0,
            )
            m01 = spool.tile([P, N], F32, tag="m01")
            nc.vector.tensor_scalar(
                m01[:], sd[jb][:], 0.0, None,
                op0=mybir.AluOpType.not_equal,
            )
            nc.vector.tensor_mul(e[:], e[:], m01[:])
            em.append(e)

        # agg^T accumulation: out[i, 0:768] = sum_j e[j, i] * feat[j, :],
        # col 768 = denominator
        for ib in range(NB):
            agg = agg_ps.tile([P, D + 1], F32, tag="agg")
            for jb in range(NB):
                for h, (c0, c1) in enumerate(((0, 512), (512, D + 1))):
                    nc.tensor.matmul(
                        agg[:, c0:c1],
                        em[jb][:, ib * P:(ib + 1) * P],
                        featp[jb][:, c0:c1],
                        start=(jb == 0),
                        stop=(jb == NB - 1),
                    )
            # denom, update mask, reciprocal
            den = opool.tile([P, 4], F32, tag="den")
            # den[:,0] = max(denom, tiny); den[:,1] = (denom > 0)
            nc.vector.tensor_scalar(
                den[:, 0:1], agg[:, D:D + 1], 1e-30, None,
                op0=mybir.AluOpType.max,
            )
            nc.vector.tensor_scalar(
                den[:, 1:2], agg[:, D:D + 1], 0.0, None,
                op0=mybir.AluOpType.is_gt,
            )
            nc.vector.reciprocal(den[:, 2:3], den[:, 0:1])
            # u = aspect * rowmask ;  scale_r = u / denom ; um1 = 1 - u
            u = opool.tile([P, 3], F32, tag="u")
            nc.vector.tensor_mul(u[:, 0:1], den[:, 1:2], aspf_sb[:, ib:ib + 1])
            nc.vector.tensor_mul(u[:, 1:2], u[:, 0:1], den[:, 2:3])
            nc.vector.tensor_scalar(
                u[:, 2:3], u[:, 0:1], -1.0, 1.0,
                op0=mybir.AluOpType.mult, op1=mybir.AluOpType.add,
            )
            # out = agg * scale_r + features * (1 - u)
            o1 = opool.tile([P, D], F32, tag="o1")
            nc.scalar.mul(o1[:], agg[:, 0:D], u[:, 1:2])
            o2 = opool.tile([P, D], F32, tag="o2")
            nc.scalar.mul(o2[:], featp[ib][:, 0:D], u[:, 2:3])
            nc.vector.tensor_add(o1[:], o1[:], o2[:])
            nc.sync.dma_start(out[ib * P:(ib + 1) * P, :], o1[:])

    nc.finalize()
    return nc


def _get_nc():
    if "nc" not in _CACHED:
        _CACHED["nc"] = _build()
    return _CACHED["nc"]


def kernel(features, aspect_onehot, adj_matrix, w_att):
    features = np.ascontiguousarray(features, dtype=np.float32)
    adj_matrix = np.ascontiguousarray(adj_matrix, dtype=np.float32)
    w_att = np.asarray(w_att, dtype=np.float32)
    B = features.shape[0]

    w_dep = w_att[D:D + DEP]
    wpad = np.zeros((P, 126 + P), dtype=np.float32)
    wpad[0:DEP, 126] = w_dep
    wpad[DEP:2 * DEP, 127] = w_dep
    ident = np.eye(P, dtype=np.float32)
    aspf = aspect_onehot.astype(np.float32)

    nc = _get_nc()
    in_maps = [
        {
            "adj": adj_matrix[b].reshape(N, N * DEP),
            "feat": features[b],
            "aspf": aspf[b],
            "ident": ident,
            "wpad": wpad,
            "wnbr": w_att[0:D].copy(),
        }
        for b in range(B)
    ]
    res = run_bass_kernel_spmd(nc, in_maps, list(range(B)))
    return np.stack([res.results[b]["out"] for b in range(B)], axis=0)

